# revision 3
# baseline (speedup 1.0000x reference)
"""Trainium2 Bass kernel: GroupEmbeddingBag(10M x 3, mean) + 3-layer linear MLP.

Reference computation:
    eb  = embedding_bag_mean(emb_weight, eb_input, eb_offset)      # [B, 3]
    mlp = ((x @ w0.T + b0) @ w1.T + b1) @ w2.T + b2                # [B, 3]
    out = concat([eb, eb, eb, mlp], axis=1)                        # [B, 12]

Sharding: the embedding table is replicated to each core's HBM and the 16384
bags are split 8 ways (2048 bags / core, 50 indices each -> 102400 row
gathers per core via gpsimd indirect DMA).  Output rows are disjoint per
core, so no collectives are needed; the host just concatenates.

Index layout: host pre-permutes indices so SBUF partition p holds the bags
{j*128+p : j in 0..15} -> the gathered rows land as [128, 16*GS*3] and the
per-bag mean is a strided DVE tensor_reduce.  The MLP (no nonlinearity) runs
transposed: x is PE-transposed chunkwise, then three chained matmuls produce
y2.T, which is PE-transposed back and interleaved into the output tile.
"""

import numpy as np

NUM_EMB = 10_000_000
EMB_DIM = 3
B = 16384
HIST = 50
K_IN = 128
N_CORES = 8
P = 128
BAGS_PER_CORE = B // N_CORES        # 2048
JSLOTS = BAGS_PER_CORE // P         # 16
NCHUNK = 8                          # indirect-gather chunks (for overlap)

_KERNEL_CACHE = {}
LAST_RESULT = None                  # BassKernelResults of the last run (for test harness)


def build_kernel(num_emb=NUM_EMB, gs=HIST, pad_mode=False, nchunk=NCHUNK):
    import concourse.bass as bass
    import concourse.mybir as mybir
    import concourse.tile as tile
    from concourse import bacc
    from concourse.masks import make_identity
    from contextlib import ExitStack

    f32 = mybir.dt.float32
    i32 = mybir.dt.int32
    Ident = mybir.ActivationFunctionType.Identity

    nbags = BAGS_PER_CORE
    nsamp = BAGS_PER_CORE
    nidx = JSLOTS * gs
    XCH = nsamp // P                # 16 x/y2 transpose chunks
    NTILE = 512
    NT = nsamp // NTILE             # 4 matmul N tiles
    jper = JSLOTS // nchunk         # j-slots per gather chunk

    nc = bacc.Bacc(
        "TRN2",
        target_bir_lowering=False,
        debug=False,
        enable_asserts=False,
        num_devices=N_CORES,
    )

    emb = nc.dram_tensor("emb", [num_emb, EMB_DIM], f32, kind="ExternalInput")
    idx = nc.dram_tensor("idx", [P, nidx], i32, kind="ExternalInput")
    x = nc.dram_tensor("x", [nsamp, K_IN], f32, kind="ExternalInput")
    w0 = nc.dram_tensor("w0", [12, K_IN], f32, kind="ExternalInput")
    w1 = nc.dram_tensor("w1", [6, 12], f32, kind="ExternalInput")
    w2 = nc.dram_tensor("w2", [3, 6], f32, kind="ExternalInput")
    b0 = nc.dram_tensor("b0", [12, 1], f32, kind="ExternalInput")
    b1 = nc.dram_tensor("b1", [6, 1], f32, kind="ExternalInput")
    b2 = nc.dram_tensor("b2", [3, 1], f32, kind="ExternalInput")
    scale = nc.dram_tensor("scale", [P, JSLOTS], f32, kind="ExternalInput")
    out = nc.dram_tensor("out", [nbags, 12], f32, kind="ExternalOutput")

    with tile.TileContext(nc) as tc, ExitStack() as ctx:
        const = ctx.enter_context(tc.tile_pool(name="const", bufs=1))
        sb = ctx.enter_context(tc.tile_pool(name="sb", bufs=1))
        gpool = ctx.enter_context(tc.tile_pool(name="gpool", bufs=nchunk))
        psx = ctx.enter_context(tc.tile_pool(name="psx", bufs=3, space="PSUM"))
        psy = ctx.enter_context(tc.tile_pool(name="psy", bufs=4, space="PSUM"))
        psb = ctx.enter_context(tc.tile_pool(name="psb", bufs=1, space="PSUM"))

        # ---- input loads -------------------------------------------------
        idx_t = sb.tile([P, nidx], i32)
        nc.sync.dma_start(idx_t[:], idx[:, :])

        xbuf = sb.tile([P, XCH * K_IN], f32)
        nc.sync.dma_start(
            xbuf[:].rearrange("p (q f) -> p q f", q=XCH),
            x[:, :].rearrange("(q r) f -> r q f", r=P),
        )

        w0_t = const.tile([12, K_IN], f32)
        nc.sync.dma_start(w0_t[:], w0[:, :])
        w1_t = const.tile([6, 12], f32)
        nc.sync.dma_start(w1_t[:], w1[:, :])
        w2_t = const.tile([3, 6], f32)
        nc.sync.dma_start(w2_t[:], w2[:, :])
        b0_t = const.tile([12, 1], f32)
        nc.sync.dma_start(b0_t[:], b0[:, :])
        b1_t = const.tile([6, 1], f32)
        nc.sync.dma_start(b1_t[:], b1[:, :])
        b2_t = const.tile([3, 1], f32)
        nc.sync.dma_start(b2_t[:], b2[:, :])
        scale_t = const.tile([P, JSLOTS], f32)
        nc.sync.dma_start(scale_t[:], scale[:, :])

        id128 = const.tile([P, P], f32)
        make_identity(nc, id128[:])

        # ---- embedding gather + segment mean ----------------------------
        # HW indirect DMA consumes ONE offset per partition per instruction
        # (streams dest free-size contiguously from it), so gather 128 rows
        # per instruction: inst k reads idx_t[:, k] -> gt[:, k*3:(k+1)*3].
        ebsum = sb.tile([P, JSLOTS * EMB_DIM], f32)
        ebsum_v = ebsum[:].rearrange("p (j c) -> p j c", j=JSLOTS)
        kper = jper * gs                       # insts per chunk
        g_tiles = []
        for ch in range(nchunk):
            gt = gpool.tile([P, kper * EMB_DIM], f32, tag="g")
            if pad_mode:
                nc.vector.memset(gt[:], 0.0)
            for k in range(kper):
                kk = ch * kper + k
                nc.gpsimd.indirect_dma_start(
                    out=gt[:, k * EMB_DIM : (k + 1) * EMB_DIM],
                    out_offset=None,
                    in_=emb[:, :],
                    in_offset=bass.IndirectOffsetOnAxis(
                        ap=idx_t[:, kk : kk + 1],
                        axis=0,
                    ),
                    bounds_check=(num_emb - 1) if pad_mode else None,
                    oob_is_err=not pad_mode,
                )
            g_tiles.append(gt)

        for ch in range(nchunk):
            v = g_tiles[ch][:].rearrange(
                "p (j k c) -> p j c k", j=jper, k=gs, c=EMB_DIM
            )
            nc.vector.tensor_reduce(
                out=ebsum_v[:, ch * jper : (ch + 1) * jper, :],
                in_=v,
                axis=mybir.AxisListType.X,
                op=mybir.AluOpType.add,
            )

        # out tile: [p, j, c12] ; bag (j*128+p) -> columns 0:3,3:6,6:9 = eb mean
        out_t = sb.tile([P, JSLOTS * 12], f32)
        out_v = out_t[:].rearrange("p (j c) -> p j c", j=JSLOTS)
        sc3 = scale_t[:].unsqueeze(2).to_broadcast([P, JSLOTS, EMB_DIM])
        for r in range(3):
            nc.vector.tensor_tensor(
                out=out_v[:, :, r * 3 : (r + 1) * 3],
                in0=ebsum_v[:, :, :],
                in1=sc3,
                op=mybir.AluOpType.mult,
            )

        # ---- MLP (transposed) -------------------------------------------
        # x.T: PE-transpose each [128,128] chunk
        xT = sb.tile([P, nsamp], f32)
        xbuf_v = xbuf[:].rearrange("p (q f) -> p q f", q=XCH)
        for q in range(XCH):
            pt = psx.tile([P, P], f32, tag="pt")
            nc.tensor.transpose(out=pt[:], in_=xbuf_v[:, q, :], identity=id128[:])
            nc.vector.tensor_copy(out=xT[:, q * P : (q + 1) * P], in_=pt[:])

        # weight transposes (tiny)
        w0T = const.tile([P, 12], f32)
        pw0 = psx.tile([P, 12], f32, tag="pt")
        nc.tensor.transpose(out=pw0[:], in_=w0_t[:], identity=id128[:12, :12])
        nc.vector.tensor_copy(out=w0T[:], in_=pw0[:])

        w1T = const.tile([12, 6], f32)
        pw1 = psx.tile([12, 6], f32, tag="pt")
        nc.tensor.transpose(out=pw1[:], in_=w1_t[:], identity=id128[:6, :6])
        nc.vector.tensor_copy(out=w1T[:], in_=pw1[:])

        w2T = const.tile([6, 3], f32)
        pw2 = psx.tile([6, 3], f32, tag="pt")
        nc.tensor.transpose(out=pw2[:], in_=w2_t[:], identity=id128[:3, :3])
        nc.vector.tensor_copy(out=w2T[:], in_=pw2[:])

        # chained matmuls producing y.T; bias added during PSUM->SBUF copy on ACT
        y0T = sb.tile([12, nsamp], f32)
        y1T = sb.tile([6, nsamp], f32)
        y2T = sb.tile([3, nsamp], f32)
        for t in range(NT):
            sl = slice(t * NTILE, (t + 1) * NTILE)
            p0 = psy.tile([12, NTILE], f32, tag="py")
            nc.tensor.matmul(out=p0[:], lhsT=w0T[:], rhs=xT[:, sl], start=True, stop=True)
            nc.scalar.activation(out=y0T[:, sl], in_=p0[:], func=Ident, bias=b0_t[:, :1])
            p1 = psy.tile([6, NTILE], f32, tag="py")
            nc.tensor.matmul(out=p1[:], lhsT=w1T[:], rhs=y0T[:, sl], start=True, stop=True)
            nc.scalar.activation(out=y1T[:, sl], in_=p1[:], func=Ident, bias=b1_t[:, :1])
            p2 = psy.tile([3, NTILE], f32, tag="py")
            nc.tensor.matmul(out=p2[:], lhsT=w2T[:], rhs=y1T[:, sl], start=True, stop=True)
            nc.scalar.activation(out=y2T[:, sl], in_=p2[:], func=Ident, bias=b2_t[:, :1])

        # back-transpose y2.T chunks into [128, 16*3] PSUM, then into out tile
        yc = psb.tile([P, XCH * EMB_DIM], f32)
        for t in range(XCH):
            nc.tensor.transpose(
                out=yc[:, t * 3 : (t + 1) * 3],
                in_=y2T[:, t * P : (t + 1) * P],
                identity=id128[:3, :3],
            )
        nc.vector.tensor_copy(
            out=out_v[:, :, 9:12],
            in_=yc[:].rearrange("p (j c) -> p j c", j=XCH),
        )

        # ---- store: bag (j*128+p) -> dram row j*128+p -------------------
        nc.sync.dma_start(
            out[:, :].rearrange("(j p) c -> p j c", p=P),
            out_v,
        )

    nc.compile()
    return nc


def _get_kernel(num_emb, gs, pad_mode):
    key = (num_emb, gs, pad_mode)
    if key not in _KERNEL_CACHE:
        _KERNEL_CACHE[key] = build_kernel(num_emb=num_emb, gs=gs, pad_mode=pad_mode)
    return _KERNEL_CACHE[key]


def _prepare(inputs, num_emb=NUM_EMB):
    """Host-side sharding: returns (gs, pad_mode, in_maps)."""
    eb_input = np.asarray(inputs["eb_input"]).astype(np.int64, copy=False)
    eb_offset = np.asarray(inputs["eb_offset"]).astype(np.int64, copy=False)
    mlp_input = np.asarray(inputs["mlp_input"], dtype=np.float32)
    emb_weight = np.ascontiguousarray(np.asarray(inputs["emb_weight"], dtype=np.float32))
    w0 = np.ascontiguousarray(np.asarray(inputs["w0"], dtype=np.float32))
    w1 = np.ascontiguousarray(np.asarray(inputs["w1"], dtype=np.float32))
    w2 = np.ascontiguousarray(np.asarray(inputs["w2"], dtype=np.float32))
    b0 = np.asarray(inputs["b0"], dtype=np.float32).reshape(12, 1)
    b1 = np.asarray(inputs["b1"], dtype=np.float32).reshape(6, 1)
    b2 = np.asarray(inputs["b2"], dtype=np.float32).reshape(3, 1)

    n = eb_input.shape[0]
    assert eb_offset.shape[0] == B and mlp_input.shape == (B, K_IN)
    counts = np.diff(np.concatenate([eb_offset, [n]]))

    if n == B * HIST and np.all(counts == HIST):
        gs, pad_mode = HIST, False
        idx_all = eb_input.astype(np.int32).reshape(B, HIST)
    else:
        gs, pad_mode = int(max(int(counts.max()), 1)), True
        idx_all = np.full((B, gs), num_emb + 1, dtype=np.int32)
        pos = np.arange(gs)[None, :] < counts[:, None]          # [B, gs] valid mask
        src = eb_offset[:, None] + np.arange(gs)[None, :]
        idx_all[pos] = eb_input[src[pos]]

    scales = (1.0 / np.maximum(counts, 1)).astype(np.float32)

    in_maps = []
    for c in range(N_CORES):
        lo = c * BAGS_PER_CORE
        hi = lo + BAGS_PER_CORE
        # bag (local b = j*128+p) -> partition p, slot j
        idx_c = (
            idx_all[lo:hi]
            .reshape(JSLOTS, P, gs)
            .transpose(1, 0, 2)
            .reshape(P, JSLOTS * gs)
        )
        sc_c = scales[lo:hi].reshape(JSLOTS, P).T
        in_maps.append(
            {
                "emb": emb_weight,
                "idx": np.ascontiguousarray(idx_c),
                "x": np.ascontiguousarray(mlp_input[lo:hi]),
                "w0": w0, "w1": w1, "w2": w2,
                "b0": b0, "b1": b1, "b2": b2,
                "scale": np.ascontiguousarray(sc_c),
            }
        )
    return gs, pad_mode, in_maps


def kernel(**inputs) -> np.ndarray:
    global LAST_RESULT
    from concourse.bass_utils import run_bass_kernel_spmd

    gs, pad_mode, in_maps = _prepare(inputs)
    nc = _get_kernel(NUM_EMB, gs, pad_mode)
    try:
        res = run_bass_kernel_spmd(nc, in_maps, core_ids=list(range(N_CORES)))
    except ModuleNotFoundError:
        # BASS_TRACE was requested but this image lacks the axon NTFF hook
        # plumbing; rerun without tracing.
        import os

        os.environ["BASS_NEVER_TRACE"] = "1"
        res = run_bass_kernel_spmd(nc, in_maps, core_ids=list(range(N_CORES)))
    LAST_RESULT = res
    return np.concatenate([r["out"] for r in res.results], axis=0)


# revision 6
# speedup vs baseline: 1.0064x; 1.0064x over previous
"""Trainium2 Bass kernel: GroupEmbeddingBag(10M x 3, mean) + 3-layer linear MLP.

Reference computation:
    eb  = embedding_bag_mean(emb_weight, eb_input, eb_offset)      # [B, 3]
    mlp = ((x @ w0.T + b0) @ w1.T + b1) @ w2.T + b2                # [B, 3]
    out = concat([eb, eb, eb, mlp], axis=1)                        # [B, 12]

Sharding: the embedding table is replicated to each core's HBM and the 16384
bags are split 8 ways (2048 bags / core, 50 indices each -> 102400 row
gathers per core via gpsimd indirect DMA).  Output rows are disjoint per
core, so no collectives are needed; the host just concatenates.

Index layout: host pre-permutes indices so SBUF partition p holds the bags
{j*128+p : j in 0..15} -> the gathered rows land as [128, 16*GS*3] and the
per-bag mean is a strided DVE tensor_reduce.  The MLP (no nonlinearity) runs
transposed: x is PE-transposed chunkwise, then three chained matmuls produce
y2.T, which is PE-transposed back and interleaved into the output tile.
"""

import numpy as np

NUM_EMB = 10_000_000
EMB_DIM = 3
B = 16384
HIST = 50
K_IN = 128
N_CORES = 8
P = 128
BAGS_PER_CORE = B // N_CORES        # 2048
JSLOTS = BAGS_PER_CORE // P         # 16
NCHUNK = 8                          # indirect-gather chunks (for overlap)

_KERNEL_CACHE = {}
LAST_RESULT = None                  # BassKernelResults of the last run (for test harness)


def build_kernel(num_emb=NUM_EMB, gs=HIST, pad_mode=False, nchunk=NCHUNK):
    import concourse.bass as bass
    import concourse.mybir as mybir
    import concourse.tile as tile
    from concourse import bacc
    from concourse.masks import make_identity
    from contextlib import ExitStack

    f32 = mybir.dt.float32
    i32 = mybir.dt.int32
    Ident = mybir.ActivationFunctionType.Identity

    nbags = BAGS_PER_CORE
    nsamp = BAGS_PER_CORE
    nidx = JSLOTS * gs
    XCH = nsamp // P                # 16 x/y2 transpose chunks
    NTILE = 512
    NT = nsamp // NTILE             # 4 matmul N tiles
    jper = JSLOTS // nchunk         # j-slots per gather chunk

    nc = bacc.Bacc(
        "TRN2",
        target_bir_lowering=False,
        debug=False,
        enable_asserts=False,
        num_devices=N_CORES,
    )

    emb = nc.dram_tensor("emb", [num_emb, EMB_DIM], f32, kind="ExternalInput")
    idx = nc.dram_tensor("idx", [P, nidx], i32, kind="ExternalInput")
    x = nc.dram_tensor("x", [nsamp, K_IN], f32, kind="ExternalInput")
    w0 = nc.dram_tensor("w0", [12, K_IN], f32, kind="ExternalInput")
    w1 = nc.dram_tensor("w1", [6, 12], f32, kind="ExternalInput")
    w2 = nc.dram_tensor("w2", [3, 6], f32, kind="ExternalInput")
    b0 = nc.dram_tensor("b0", [12, 1], f32, kind="ExternalInput")
    b1 = nc.dram_tensor("b1", [6, 1], f32, kind="ExternalInput")
    b2 = nc.dram_tensor("b2", [3, 1], f32, kind="ExternalInput")
    scale = nc.dram_tensor("scale", [P, JSLOTS], f32, kind="ExternalInput")
    out = nc.dram_tensor("out", [nbags, 12], f32, kind="ExternalOutput")

    with tile.TileContext(nc) as tc, ExitStack() as ctx:
        const = ctx.enter_context(tc.tile_pool(name="const", bufs=1))
        sb = ctx.enter_context(tc.tile_pool(name="sb", bufs=1))
        gpool = ctx.enter_context(tc.tile_pool(name="gpool", bufs=nchunk))
        psx = ctx.enter_context(tc.tile_pool(name="psx", bufs=3, space="PSUM"))
        psy = ctx.enter_context(tc.tile_pool(name="psy", bufs=4, space="PSUM"))
        psb = ctx.enter_context(tc.tile_pool(name="psb", bufs=1, space="PSUM"))

        # ---- input loads -------------------------------------------------
        idx_t = sb.tile([P, nidx], i32)
        nc.sync.dma_start(idx_t[:], idx[:, :])

        xbuf = sb.tile([P, XCH * K_IN], f32)
        nc.sync.dma_start(
            xbuf[:].rearrange("p (q f) -> p q f", q=XCH),
            x[:, :].rearrange("(q r) f -> r q f", r=P),
        )

        w0_t = const.tile([12, K_IN], f32)
        nc.sync.dma_start(w0_t[:], w0[:, :])
        w1_t = const.tile([6, 12], f32)
        nc.sync.dma_start(w1_t[:], w1[:, :])
        w2_t = const.tile([3, 6], f32)
        nc.sync.dma_start(w2_t[:], w2[:, :])
        b0_t = const.tile([12, 1], f32)
        nc.sync.dma_start(b0_t[:], b0[:, :])
        b1_t = const.tile([6, 1], f32)
        nc.sync.dma_start(b1_t[:], b1[:, :])
        b2_t = const.tile([3, 1], f32)
        nc.sync.dma_start(b2_t[:], b2[:, :])
        scale_t = const.tile([P, JSLOTS], f32)
        nc.sync.dma_start(scale_t[:], scale[:, :])

        id128 = const.tile([P, P], f32)
        make_identity(nc, id128[:])

        # ---- embedding gather + segment mean ----------------------------
        # HW indirect DMA consumes ONE offset per partition per instruction
        # (streams dest free-size contiguously from it), so gather 128 rows
        # per instruction: inst k reads idx_t[:, k] -> gt[:, k*3:(k+1)*3].
        ebsum = sb.tile([P, JSLOTS * EMB_DIM], f32)
        ebsum_v = ebsum[:].rearrange("p (j c) -> p j c", j=JSLOTS)
        kper = jper * gs                       # insts per chunk
        g_tiles = []
        for ch in range(nchunk):
            gt = gpool.tile([P, kper * EMB_DIM], f32, tag="g")
            if pad_mode:
                nc.vector.memset(gt[:], 0.0)
            for k in range(kper):
                kk = ch * kper + k
                nc.gpsimd.indirect_dma_start(
                    out=gt[:, k * EMB_DIM : (k + 1) * EMB_DIM],
                    out_offset=None,
                    in_=emb[:, :],
                    in_offset=bass.IndirectOffsetOnAxis(
                        ap=idx_t[:, kk : kk + 1],
                        axis=0,
                    ),
                    bounds_check=(num_emb - 1) if pad_mode else None,
                    oob_is_err=not pad_mode,
                )
            g_tiles.append(gt)

        for ch in range(nchunk):
            v = g_tiles[ch][:].rearrange(
                "p (j k c) -> p j c k", j=jper, k=gs, c=EMB_DIM
            )
            nc.vector.tensor_reduce(
                out=ebsum_v[:, ch * jper : (ch + 1) * jper, :],
                in_=v,
                axis=mybir.AxisListType.X,
                op=mybir.AluOpType.add,
            )

        # out tile: [p, j, c12] ; bag (j*128+p) -> columns 0:3,3:6,6:9 = eb mean
        out_t = sb.tile([P, JSLOTS * 12], f32)
        out_v = out_t[:].rearrange("p (j c) -> p j c", j=JSLOTS)
        sc3 = scale_t[:].unsqueeze(2).to_broadcast([P, JSLOTS, EMB_DIM])
        for r in range(3):
            nc.vector.tensor_tensor(
                out=out_v[:, :, r * 3 : (r + 1) * 3],
                in0=ebsum_v[:, :, :],
                in1=sc3,
                op=mybir.AluOpType.mult,
            )

        # ---- MLP (transposed) -------------------------------------------
        # x.T: PE-transpose each [128,128] chunk
        xT = sb.tile([P, nsamp], f32)
        xbuf_v = xbuf[:].rearrange("p (q f) -> p q f", q=XCH)
        for q in range(XCH):
            pt = psx.tile([P, P], f32, tag="pt")
            nc.tensor.transpose(out=pt[:], in_=xbuf_v[:, q, :], identity=id128[:])
            nc.vector.tensor_copy(out=xT[:, q * P : (q + 1) * P], in_=pt[:])

        # weight transposes (tiny)
        w0T = const.tile([P, 12], f32)
        pw0 = psx.tile([P, 12], f32, tag="pt")
        nc.tensor.transpose(out=pw0[:], in_=w0_t[:], identity=id128[:12, :12])
        nc.vector.tensor_copy(out=w0T[:], in_=pw0[:])

        w1T = const.tile([12, 6], f32)
        pw1 = psx.tile([12, 6], f32, tag="pt")
        nc.tensor.transpose(out=pw1[:], in_=w1_t[:], identity=id128[:6, :6])
        nc.vector.tensor_copy(out=w1T[:], in_=pw1[:])

        w2T = const.tile([6, 3], f32)
        pw2 = psx.tile([6, 3], f32, tag="pt")
        nc.tensor.transpose(out=pw2[:], in_=w2_t[:], identity=id128[:3, :3])
        nc.vector.tensor_copy(out=w2T[:], in_=pw2[:])

        # chained matmuls producing y.T; bias added during PSUM->SBUF copy on ACT
        y0T = sb.tile([12, nsamp], f32)
        y1T = sb.tile([6, nsamp], f32)
        y2T = sb.tile([3, nsamp], f32)
        for t in range(NT):
            sl = slice(t * NTILE, (t + 1) * NTILE)
            p0 = psy.tile([12, NTILE], f32, tag="py")
            nc.tensor.matmul(out=p0[:], lhsT=w0T[:], rhs=xT[:, sl], start=True, stop=True)
            nc.scalar.activation(out=y0T[:, sl], in_=p0[:], func=Ident, bias=b0_t[:, :1])
            p1 = psy.tile([6, NTILE], f32, tag="py")
            nc.tensor.matmul(out=p1[:], lhsT=w1T[:], rhs=y0T[:, sl], start=True, stop=True)
            nc.scalar.activation(out=y1T[:, sl], in_=p1[:], func=Ident, bias=b1_t[:, :1])
            p2 = psy.tile([3, NTILE], f32, tag="py")
            nc.tensor.matmul(out=p2[:], lhsT=w2T[:], rhs=y1T[:, sl], start=True, stop=True)
            nc.scalar.activation(out=y2T[:, sl], in_=p2[:], func=Ident, bias=b2_t[:, :1])

        # back-transpose y2.T chunks into [128, 16*3] PSUM, then into out tile
        yc = psb.tile([P, XCH * EMB_DIM], f32)
        for t in range(XCH):
            nc.tensor.transpose(
                out=yc[:, t * 3 : (t + 1) * 3],
                in_=y2T[:, t * P : (t + 1) * P],
                identity=id128[:3, :3],
            )
        nc.vector.tensor_copy(
            out=out_v[:, :, 9:12],
            in_=yc[:].rearrange("p (j c) -> p j c", j=XCH),
        )

        # ---- store: bag (j*128+p) -> dram row j*128+p -------------------
        nc.sync.dma_start(
            out[:, :].rearrange("(j p) c -> p j c", p=P),
            out_v,
        )

    nc.compile()
    return nc


def _get_kernel(num_emb, gs, pad_mode):
    key = (num_emb, gs, pad_mode)
    if key not in _KERNEL_CACHE:
        _KERNEL_CACHE[key] = build_kernel(num_emb=num_emb, gs=gs, pad_mode=pad_mode)
    return _KERNEL_CACHE[key]


def _prepare(inputs, num_emb=NUM_EMB):
    """Host-side sharding: returns (gs, pad_mode, in_maps)."""
    eb_input = np.asarray(inputs["eb_input"]).astype(np.int64, copy=False)
    eb_offset = np.asarray(inputs["eb_offset"]).astype(np.int64, copy=False)
    mlp_input = np.asarray(inputs["mlp_input"], dtype=np.float32)
    emb_weight = np.ascontiguousarray(np.asarray(inputs["emb_weight"], dtype=np.float32))
    w0 = np.ascontiguousarray(np.asarray(inputs["w0"], dtype=np.float32))
    w1 = np.ascontiguousarray(np.asarray(inputs["w1"], dtype=np.float32))
    w2 = np.ascontiguousarray(np.asarray(inputs["w2"], dtype=np.float32))
    b0 = np.asarray(inputs["b0"], dtype=np.float32).reshape(12, 1)
    b1 = np.asarray(inputs["b1"], dtype=np.float32).reshape(6, 1)
    b2 = np.asarray(inputs["b2"], dtype=np.float32).reshape(3, 1)

    n = eb_input.shape[0]
    assert eb_offset.shape[0] == B and mlp_input.shape == (B, K_IN)
    counts = np.diff(np.concatenate([eb_offset, [n]]))

    if n == B * HIST and np.all(counts == HIST):
        gs, pad_mode = HIST, False
        idx_all = eb_input.astype(np.int32).reshape(B, HIST)
    else:
        gs, pad_mode = int(max(int(counts.max()), 1)), True
        idx_all = np.full((B, gs), num_emb + 1, dtype=np.int32)
        pos = np.arange(gs)[None, :] < counts[:, None]          # [B, gs] valid mask
        src = eb_offset[:, None] + np.arange(gs)[None, :]
        idx_all[pos] = eb_input[src[pos]]

    scales = (1.0 / np.maximum(counts, 1)).astype(np.float32)

    in_maps = []
    for c in range(N_CORES):
        lo = c * BAGS_PER_CORE
        hi = lo + BAGS_PER_CORE
        # bag (local b = j*128+p) -> partition p, slot j
        idx_c = (
            idx_all[lo:hi]
            .reshape(JSLOTS, P, gs)
            .transpose(1, 0, 2)
            .reshape(P, JSLOTS * gs)
        )
        sc_c = scales[lo:hi].reshape(JSLOTS, P).T
        in_maps.append(
            {
                "emb": emb_weight,
                "idx": np.ascontiguousarray(idx_c),
                "x": np.ascontiguousarray(mlp_input[lo:hi]),
                "w0": w0, "w1": w1, "w2": w2,
                "b0": b0, "b1": b1, "b2": b2,
                "scale": np.ascontiguousarray(sc_c),
            }
        )
    return gs, pad_mode, in_maps


def kernel(**inputs) -> np.ndarray:
    global LAST_RESULT
    from concourse.bass_utils import run_bass_kernel_spmd

    gs, pad_mode, in_maps = _prepare(inputs)
    nc = _get_kernel(NUM_EMB, gs, pad_mode)
    try:
        res = run_bass_kernel_spmd(nc, in_maps, core_ids=list(range(N_CORES)))
    except ModuleNotFoundError:
        # BASS_TRACE was requested but this image lacks the axon NTFF hook
        # plumbing; rerun without tracing.
        import os

        os.environ["BASS_NEVER_TRACE"] = "1"
        res = run_bass_kernel_spmd(nc, in_maps, core_ids=list(range(N_CORES)))
    LAST_RESULT = res
    return np.concatenate([r["out"] for r in res.results], axis=0)
